# revision 13
# baseline (speedup 1.0000x reference)
"""Trainium2 Bass kernel for BiAttention (b=8, n=m=1024, d=512).

Sharding: data-parallel over batch — one batch element per NeuronCore,
8 cores, no cross-core communication.

Per-core algorithm (softmax shift-invariance folds the Linear(3d,1)
row/col terms, bias, and both padding masks into per-row/col exponent
weights g1 = exp(s1+logm1), g2 = exp(s2+logm2); logits ~ N(0,1) so raw
exp is safe):

  sim      = (x1*w3) @ x2^T              (n, m)   [tri term only]
  E        = exp(sim)                    bf16
  ET8      = fp8(E^T)                    via DMA xbar transpose + DVE cast
  U_row    = ET8^T @ (x2*g2/4)  -> c2q = U_row/den1,  den1 = g2c8 @ ET8
  U_col    = E^T   @ (x1*g1)    -> q2c = U_col/den2,  den2 = g1cb @ E
  V        = ET8^T @ Q2C        -> q2c_att = V * rden1/SQ
  out      = [x1, c2q, x1*c2q, x1*q2c_att]        (n, 4d)

Precision: sim/U_col in bf16, U_row/V in fp8e4 DoubleRow (2 contraction
tiles per instruction), f32 PSUM accumulation, exact f32 softmax
divisions.  Measured end-to-end rel err ~3e-3 (gate 2e-2).

Engine/DMA layout (calibrated on HW):
  - PE transposes x1/x2 directly from f32 (2 cyc/row) during the load
    phase — keeps the HAM clock warm and removes conversion latency; the
    mid-kernel E transposes use the DMA xbar (bf16) instead so the PE
    stays on matmuls.
  - PSUM evictions split Act/DVE by phase; gpsimd (no PSUM access,
    ~2.5 ns/el) only gets SBUF->SBUF scales/muls in its idle phases.
  - Loads split across the SP and Act HWDGE queues (a single queue
    sustains only ~133 GB/s on the 2 KB-granule input layout).  Output
    blocks 1-3 are staged contiguously per tile (6 KB/partition rows)
    and stored after an interleaved U_row/V loop, alternating queues.

Mask-suffix specialization: 128-row tiles fully masked at the end of
either sequence are skipped in the contractions (host inspects masks and
dispatches to a NEFF compiled for that (kn, km)); partially-masked tiles
are exact via the exponent weights.
"""

import numpy as np
from contextlib import ExitStack

import concourse.bacc as bacc
import concourse.tile as tile
import concourse.mybir as mybir
from concourse.bass_utils import run_bass_kernel_spmd
from concourse.masks import make_identity

F32 = mybir.dt.float32
BF16 = mybir.dt.bfloat16
F8 = mybir.dt.float8e4
U8 = mybir.dt.uint8
EXP = mybir.ActivationFunctionType.Exp
COPY = mybir.ActivationFunctionType.Copy
DR = mybir.MatmulPerfMode.DoubleRow

P = 128
N = 1024          # x1 rows
M = 1024          # x2 rows
D = 512           # feature dim
NT, MT, DC = N // P, M // P, D // P
NEGB = -30000.0   # exp(x + NEGB) == 0.0 exactly for |x| < 80
SX = 32.0         # x1w3 prescale (keeps bf16 products well-scaled)
SQ = 1.0          # q2c fp8 denormal error is negligible (~0.1% global)
LN4 = 1.3862943611198906

N_CORES = 8

_CACHE = {}


def _chunks(width, step=512):
    out = []
    o = 0
    while o < width:
        w = min(step, width - o)
        out.append((o, w))
        o += w
    return out


def _pairs(k):
    """(k0, is_pair) covering range(k) with DoubleRow pairs + odd tail."""
    out = [(2 * i, True) for i in range(k // 2)]
    if k % 2:
        out.append((k - 1, False))
    return out


def _build(kn, km):
    vm = km * P
    nc = bacc.Bacc("TRN2", target_bir_lowering=False, debug=False)
    x1d = nc.dram_tensor("x1", [P, NT * D], F32, kind="ExternalInput").ap()
    x2d = nc.dram_tensor("x2", [P, MT * D], F32, kind="ExternalInput").ap()
    m1d = nc.dram_tensor("x1_mask", [N], U8, kind="ExternalInput").ap()
    m2d = nc.dram_tensor("x2_mask", [M], U8, kind="ExternalInput").ap()
    wd = nc.dram_tensor("W", [3 * D], F32, kind="ExternalInput").ap()
    o12d = nc.dram_tensor("out12", [P, NT * 2 * D], F32,
                          kind="ExternalOutput").ap()
    o3d = nc.dram_tensor("out3", [P, NT * D], F32, kind="ExternalOutput").ap()

    x1r_d = x1d.rearrange("p (t d) -> p t d", t=NT)
    x2r_d = x2d.rearrange("p (t d) -> p t d", t=MT)
    o12_r = o12d.rearrange("p (t e) -> p t e", t=NT)
    o3_r = o3d.rearrange("p (t e) -> p t e", t=NT)

    with tile.TileContext(nc) as tc, ExitStack() as ctx:
        big = ctx.enter_context(tc.tile_pool(name="big", bufs=1))
        rows = ctx.enter_context(tc.tile_pool(name="rows", bufs=1))
        work = ctx.enter_context(tc.tile_pool(name="work", bufs=3))
        psb = ctx.enter_context(tc.tile_pool(name="psb", bufs=2, space="PSUM"))
        psu = ctx.enter_context(tc.tile_pool(name="psu", bufs=3, space="PSUM"))
        psd = ctx.enter_context(tc.tile_pool(name="psd", bufs=1, space="PSUM"))

        # ---------------- constants ----------------
        ident = big.tile([P, P], F32)
        make_identity(nc, ident)
        identb = big.tile([P, P], BF16)
        nc.vector.tensor_copy(identb[:], ident[:])
        negln4 = big.tile([P, 1], F32)
        nc.vector.memset(negln4[:], -LN4)

        # ---------------- DMA loads (split SP / Act queues) ----------------
        wrow = rows.tile([1, 12 * P], F32)
        nc.scalar.dma_start(wrow[:], wd.rearrange("(a n) -> a n", a=1))
        x1n = big.tile([P, NT, D], F32)
        x2n = big.tile([P, km, D], F32)
        nc.sync.dma_start(x2n[:, 0:min(4, km), :], x2r_d[:, 0:min(4, km), :])
        nc.scalar.dma_start(x1n[:, 0:4, :], x1r_d[:, 0:4, :])
        if km > 4:
            nc.sync.dma_start(x2n[:, 4:km, :], x2r_d[:, 4:km, :])
        nc.scalar.dma_start(x1n[:, 4:8, :], x1r_d[:, 4:8, :])
        m1row = rows.tile([1, N], U8)
        nc.sync.dma_start(m1row[:], m1d.rearrange("(a n) -> a n", a=1))
        m2row = rows.tile([1, M], U8)
        nc.sync.dma_start(m2row[:], m2d.rearrange("(a n) -> a n", a=1))

        # ---------------- PE warmup (keeps the HAM clock busy) -------------
        # ~10 fat dummy matmuls (512 cyc each) bridge the load wait so the
        # HAM un-throttles before the real transposes start.
        wscr = big.tile([P, D], BF16)
        nc.vector.memset(wscr[:], 0.25)
        for i in range(16):
            pw = psb.tile([P, 1024], F32, tag="ps_sim", name=f"warm{i}")
            nc.tensor.matmul(pw[:, 0:D], identb[:], wscr[:], start=True,
                             stop=True)

        # ---------------- W prep ----------------
        pwc = psd.tile([P, 16], F32, tag="small", name="pwc")
        for c in range(12):
            nc.tensor.transpose(pwc[:, c:c + 1], wrow[0:1, c * P:(c + 1) * P],
                                ident[0:1, 0:1])
        wcols = big.tile([P, 12], F32)
        nc.vector.tensor_copy(wcols[:], pwc[:, 0:12])
        w3rec = big.tile([P, 4], F32)
        nc.vector.reciprocal(w3rec[:], wcols[:, 8:12])
        u1f = big.tile([P, 4], F32)
        nc.vector.tensor_mul(u1f[:], wcols[:, 0:4], w3rec[:])
        u1r = big.tile([P, 4], BF16)       # w1/(w3*SX): recovers s1 from x1w3T
        nc.vector.tensor_scalar_mul(u1r[:], u1f[:], 1.0 / SX)
        w2r = big.tile([P, 4], BF16)
        nc.vector.tensor_copy(w2r[:], wcols[:, 4:8])
        w3s = big.tile([P, 4], F32)        # w3 * SX (x1w3T eviction scale)
        nc.vector.tensor_scalar_mul(w3s[:], wcols[:, 8:12], SX)

        logm1 = rows.tile([1, N], F32)
        nc.vector.tensor_scalar_mul(logm1[:], m1row[:], NEGB)
        logm2 = rows.tile([1, M], F32)
        nc.vector.tensor_scalar_mul(logm2[:], m2row[:], NEGB)

        # ---------------- PE transposes of x1 / x2 ----------------
        x1w3T = big.tile([P, NT, DC, P], BF16)   # (d_lo, t, c, n_lo) = x1*w3*SX
        x2T = big.tile([P, km, DC, P], BF16)     # (d_lo, u, c, m_lo)

        def xpose_group(src, dst, q, jw, c, scale, nm):
            """Transpose tiles q*4..q*4+jw of src at d-chunk c -> dst cols."""
            pq = psu.tile([P, 4 * P], F32, tag="ps_uv", name=f"xp{nm}{q}_{c}")
            for j in range(jw):
                nc.tensor.transpose(pq[:, j * P:(j + 1) * P],
                                    src[:, q * 4 + j, c * P:(c + 1) * P],
                                    ident[:])
            out_ap = dst[:, q * 4:q * 4 + jw, c, :]
            if scale is None:
                nc.scalar.activation(out_ap, pq[:, 0:jw * P], COPY)
            else:
                nc.vector.tensor_scalar_mul(out_ap, pq[:, 0:jw * P],
                                            scale[:, c:c + 1])

        def s_chunk(name, lhs, rhsT, brow, logm, off, w):
            t0, ntile = off // P, w // P
            ps_s = psd.tile([1, D], F32, tag="small", name=f"ps{name}{off}")
            for c in range(DC):
                nc.tensor.matmul(ps_s[0:1, 0:w], lhs[:, c:c + 1],
                                 rhsT[:, t0:t0 + ntile, c, :],
                                 start=(c == 0), stop=(c == DC - 1))
            nc.vector.tensor_add(brow[:, off:off + w], ps_s[0:1, 0:w],
                                 logm[:, off:off + w])

        def col_of(name, brow, nt):
            pbc = psd.tile([P, 16], F32, tag="small", name=f"pbc{name}")
            for t in range(nt):
                nc.tensor.transpose(pbc[:, t:t + 1], brow[0:1, t * P:(t + 1) * P],
                                    ident[0:1, 0:1])
            return pbc

        b1row = rows.tile([1, N], F32)
        b2row = rows.tile([1, M], F32)
        E_raw = big.tile([P, NT, vm], BF16)      # exp(sim), n-major
        ETraw = big.tile([P, NT, km, P], BF16)   # (m_lo, t, u, n_lo)
        ET8 = big.tile([P, NT, km, P], F8)
        x1aug = big.tile([P, kn, D], BF16)       # x1 * g1
        x2aug = big.tile([P, km, D], F8)         # x2 * g2/4
        mch = _chunks(vm)

        def sim_tile(t):
            ps = psb.tile([P, 1024], F32, tag="ps_sim", name=f"sim{t}")
            for off, w in mch:
                u0, nu = off // P, w // P
                for c in range(DC):
                    nc.tensor.matmul(ps[:, off:off + w],
                                     x1w3T[:, t, c, :],
                                     x2T[:, u0:u0 + nu, c, :],
                                     start=(c == 0), stop=(c == DC - 1))
            nc.scalar.activation(E_raw[:, t, :], ps[:, 0:vm], EXP, scale=1.0 / SX)
            nc.sync.dma_start_transpose(ETraw[:, t, :, :], E_raw[:, t, :])
            nc.vector.tensor_copy(ET8[:, t, :, :], ETraw[:, t, :, :])

        # x2 transposes, then s2 -> g2c4 -> x2aug while the Act head is idle
        for c in range(DC):
            xpose_group(x2n, x2T, 0, min(4, km), c, None, "x2")
        if km > 4:
            for c in range(DC):
                xpose_group(x2n, x2T, 1, km - 4, c, None, "x2")
        for off, w in _chunks(vm):
            s_chunk("b2", w2r, x2T, b2row, logm2, off, w)
        pbc2 = col_of("b2", b2row, km)
        g2c4 = big.tile([P, km], F32)     # exp(s2 + logm2 - ln4) = g2/4
        nc.scalar.activation(g2c4[:], pbc2[:, 0:km], EXP, bias=negln4[:, 0:1])
        g2c8 = big.tile([P, 8, 16], F8)
        for u in range(km):
            nc.vector.tensor_copy(g2c8[:, u, 0:1], g2c4[:, u:u + 1])
        for u in range(km):
            nc.scalar.activation(x2aug[:, u, :], x2n[:, u, :], COPY,
                                 scale=g2c4[:, u:u + 1])

        # x1 transposes quad 0, first sim tiles, quad 1, s1, rest of sim
        for c in range(DC):
            xpose_group(x1n, x1w3T, 0, 4, c, w3s, "x1")
        for t in range(4):
            sim_tile(t)
        for c in range(DC):
            xpose_group(x1n, x1w3T, 1, 4, c, w3s, "x1")
        for off, w in _chunks(N):
            s_chunk("b1", u1r, x1w3T, b1row, logm1, off, w)
        pbc1 = col_of("b1", b1row, NT)
        g1c = big.tile([P, NT], F32)      # exp(s1 + logm1)
        nc.scalar.activation(g1c[:], pbc1[:, 0:NT], EXP)
        g1cb = big.tile([P, 8, 16], BF16)
        for k in range(kn):
            nc.vector.tensor_copy(g1cb[:, k, 0:1], g1c[:, k:k + 1])
        for t in range(kn):
            nc.vector.tensor_scalar_mul(x1aug[:, t, :], x1n[:, t, :],
                                        g1c[:, t:t + 1])
        for t in range(4, NT):
            sim_tile(t)

        # ---------------- den2 -> rden2/rQ ----------------
        kp_m = _pairs(km)
        den2row = rows.tile([1, vm], F32)
        for off, w in _chunks(vm):
            ps_d = psd.tile([1, D], F32, tag="small", name=f"psden2{off}")
            for k in range(kn):
                nc.tensor.matmul(ps_d[0:1, 0:w], g1cb[:, k, 0:1],
                                 E_raw[:, k, off:off + w],
                                 start=(k == 0), stop=(k == kn - 1))
            nc.scalar.activation(den2row[:, off:off + w], ps_d[0:1, 0:w], COPY)
        pdc2 = col_of("d2", den2row, km)
        rden2 = big.tile([P, km], F32)
        nc.vector.reciprocal(rden2[:], pdc2[:, 0:km])
        rQ = big.tile([P, km], F32)          # rden2 * g2/4
        nc.vector.tensor_mul(rQ[:], rden2[:], g2c4[:])

        # ---------------- den1 -> rden1 ----------------
        den1row = rows.tile([1, N], F32)
        for t in range(NT):
            ps_d = psd.tile([1, D], F32, tag="small", name=f"psden1{t}")
            for i, (k0, pair) in enumerate(kp_m):
                last = i == len(kp_m) - 1
                if pair:
                    nc.tensor.matmul(ps_d[0:1, 0:P], g2c8[:, k0:k0 + 2, 0:1],
                                     ET8[:, t, k0:k0 + 2, :],
                                     start=(i == 0), stop=last, perf_mode=DR)
                else:
                    nc.tensor.matmul(ps_d[0:1, 0:P], g2c8[:, k0, 0:1],
                                     ET8[:, t, k0, :], start=(i == 0), stop=last)
            nc.scalar.activation(den1row[:, t * P:(t + 1) * P], ps_d[0:1, 0:P],
                                 COPY)
        pdc1 = col_of("d1", den1row, NT)
        rden1 = big.tile([P, NT], F32)
        nc.vector.reciprocal(rden1[:], pdc1[:, 0:NT])
        # x1 pre-scale overlaps U_col on DVE (blocks 2 and 3 share it since
        # SQ == 1)
        x1rd = big.tile([P, NT, D], F32)     # x1 * rden1
        for t in range(NT):
            nc.vector.tensor_scalar_mul(x1rd[:, t, :], x1n[:, t, :],
                                        rden1[:, t:t + 1])

        # ---------------- U_col -> Q2C ----------------
        Q2C = big.tile([P, km, D], F8)       # q2c * g2/4 * SQ
        for u in range(km):
            pu = psu.tile([P, D], F32, tag="ps_uv", name=f"pu{u}")
            for k in range(kn):
                nc.tensor.matmul(pu[:], E_raw[:, k, u * P:(u + 1) * P],
                                 x1aug[:, k, :], start=(k == 0),
                                 stop=(k == kn - 1))
            nc.scalar.activation(Q2C[:, u, :], pu[:], COPY, scale=rQ[:, u:u + 1])

        # ---------------- U_row -> blocks 1+2 ; V -> block 3 ----------------
        def uv_mm(ps_ap, t, rhs):
            for i, (k0, pair) in enumerate(kp_m):
                last = i == len(kp_m) - 1
                if pair:
                    nc.tensor.matmul(ps_ap, ET8[:, t, k0:k0 + 2, :],
                                     rhs[:, k0:k0 + 2, :],
                                     start=(i == 0), stop=last, perf_mode=DR)
                else:
                    nc.tensor.matmul(ps_ap, ET8[:, t, k0, :], rhs[:, k0, :],
                                     start=(i == 0), stop=last)

        for t in range(NT):
            if t % 2 == 0:
                c12 = work.tile([P, 2, 2 * D], F32, tag="ev", name=f"c12_{t}")
            half = c12[:, t % 2, :]
            if t % 2 == 0:
                pr = psu.tile([P, D], F32, tag="ps_uv", name=f"pr{t}")
            else:
                prb = psb.tile([P, 1024], F32, tag="ps_sim", name=f"prb{t}")
                pr = prb[:, 0:D]
            uv_mm(pr[:], t, x2aug)
            nc.scalar.activation(half[:, 0:D], pr[:], COPY,
                                 scale=rden1[:, t:t + 1])
            nc.vector.tensor_mul(half[:, D:2 * D], x1rd[:, t, :], pr[:])
            if t % 2 == 1:
                eng = nc.sync if t % 4 == 1 else nc.scalar
                eng.dma_start(o12_r[:, t - 1:t + 1, :], c12[:])

        for t in range(NT):
            if t % 4 == 0:
                o3t = work.tile([P, 4, D], F32, tag="o3", name=f"o3_{t}")
            if t % 2 == 0:
                pv = psu.tile([P, D], F32, tag="ps_uv", name=f"pv{t}")
            else:
                pvb = psb.tile([P, 1024], F32, tag="ps_sim", name=f"pvb{t}")
                pv = pvb[:, 0:D]
            uv_mm(pv[:], t, Q2C)
            nc.vector.tensor_mul(o3t[:, t % 4, :], x1rd[:, t, :], pv[:])
            if t % 4 == 3:
                eng = nc.sync if t == 3 else nc.scalar
                eng.dma_start(o3_r[:, t - 3:t + 1, :], o3t[:])

    nc.compile()
    return nc


def _kept_tiles(mask):
    """Tiles (of 128) up to and including the last one with any valid row."""
    valid = ~mask.astype(bool)
    any_valid = valid.reshape(valid.shape[0], -1, P).any(axis=2).any(axis=0)
    nz = np.nonzero(any_valid)[0]
    return int(nz[-1]) + 1 if len(nz) else 1


def _get_nc(kn, km):
    key = (kn, km)
    if key not in _CACHE:
        _CACHE[key] = _build(kn, km)
    return _CACHE[key]


def _run(inputs, trace=False, trace_cores=None):
    x1 = np.ascontiguousarray(np.asarray(inputs["x1"], dtype=np.float32))
    x2 = np.ascontiguousarray(np.asarray(inputs["x2"], dtype=np.float32))
    m1 = np.ascontiguousarray(np.asarray(inputs["x1_mask"]).astype(np.uint8))
    m2 = np.ascontiguousarray(np.asarray(inputs["x2_mask"]).astype(np.uint8))
    W = np.ascontiguousarray(np.asarray(inputs["W"], dtype=np.float32))
    nc = _get_nc(_kept_tiles(m1), _kept_tiles(m2))
    # partition-major device layouts: per-partition rows are 8-16 KB
    # contiguous, so each load/store is ~128 fat DMA descriptors instead of
    # thousands of 2 KB ones (a single HWDGE queue only sustains ~100 GB/s
    # on 2 KB descriptors).
    x1p = np.ascontiguousarray(
        x1.reshape(N_CORES, NT, P, D).transpose(0, 2, 1, 3).reshape(
            N_CORES, P, NT * D))
    x2p = np.ascontiguousarray(
        x2.reshape(N_CORES, MT, P, D).transpose(0, 2, 1, 3).reshape(
            N_CORES, P, MT * D))
    in_maps = [
        {"x1": x1p[i], "x2": x2p[i], "x1_mask": m1[i], "x2_mask": m2[i],
         "W": W}
        for i in range(N_CORES)
    ]
    res = run_bass_kernel_spmd(nc, in_maps, core_ids=list(range(N_CORES)),
                               trace=trace, trace_cores=trace_cores)
    # device returns blocks 1+2 and 3 (partition-major); block 0 is x1
    out = np.empty((N_CORES, N, 4 * D), dtype=np.float32)
    out[:, :, 0:D] = x1
    for i in range(N_CORES):
        d12 = res.results[i]["out12"].reshape(P, NT, 2 * D)
        out[i, :, D:3 * D] = d12.transpose(1, 0, 2).reshape(N, 2 * D)
        d3 = res.results[i]["out3"].reshape(P, NT, D)
        out[i, :, 3 * D:] = d3.transpose(1, 0, 2).reshape(N, D)
    return out, res


def kernel(x1, x1_mask, x2, x2_mask, W, bias=None, **_kw):
    # bias is mathematically irrelevant: a global additive constant cancels in
    # both softmaxes, and every output term is softmax-weighted.
    out, _ = _run({"x1": x1, "x1_mask": x1_mask, "x2": x2, "x2_mask": x2_mask,
                   "W": W})
    return out


# revision 14
# speedup vs baseline: 1.1608x; 1.1608x over previous
"""Trainium2 Bass kernel for BiAttention (b=8, n=m=1024, d=512).

Sharding: data-parallel over batch — one batch element per NeuronCore,
8 cores, no cross-core communication.

Per-core algorithm (softmax shift-invariance folds the Linear(3d,1)
row/col terms, bias, and both padding masks into per-row/col exponent
weights g1 = exp(s1+logm1), g2 = exp(s2+logm2); logits ~ N(0,1) so raw
exp is safe):

  sim      = (x1*w3) @ x2^T              (n, m)   [tri term only]
  E        = exp(sim)                    bf16
  ET8      = fp8(E^T)                    via DMA xbar transpose + DVE cast
  U_row    = ET8^T @ (x2*g2/4)  -> c2q = U_row/den1,  den1 = g2c8 @ ET8
  U_col    = E^T   @ (x1*g1)    -> q2c = U_col/den2,  den2 = g1cb @ E
  V        = ET8^T @ Q2C        -> q2c_att = V * rden1/SQ
  out      = [x1, c2q, x1*c2q, x1*q2c_att]        (n, 4d)

Precision: sim/U_col in bf16, U_row/V in fp8e4 DoubleRow (2 contraction
tiles per instruction), f32 PSUM accumulation, exact f32 softmax
divisions.  Measured end-to-end rel err ~3e-3 (gate 2e-2).

Engine/DMA layout (calibrated on HW):
  - PE transposes x1/x2 directly from f32 (2 cyc/row) during the load
    phase — keeps the HAM clock warm and removes conversion latency; the
    mid-kernel E transposes use the DMA xbar (bf16) instead so the PE
    stays on matmuls.
  - PSUM evictions split Act/DVE by phase; gpsimd (no PSUM access,
    ~2.5 ns/el) only gets SBUF->SBUF scales/muls in its idle phases.
  - Loads split across the SP and Act HWDGE queues (a single queue
    sustains only ~133 GB/s on the 2 KB-granule input layout).  Output
    blocks 1-3 are staged contiguously per tile (6 KB/partition rows)
    and stored after an interleaved U_row/V loop, alternating queues.

Mask-suffix specialization: 128-row tiles fully masked at the end of
either sequence are skipped in the contractions (host inspects masks and
dispatches to a NEFF compiled for that (kn, km)); partially-masked tiles
are exact via the exponent weights.
"""

import numpy as np
from contextlib import ExitStack

import concourse.bacc as bacc
import concourse.tile as tile
import concourse.mybir as mybir
from concourse.bass_utils import run_bass_kernel_spmd
from concourse.masks import make_identity

F32 = mybir.dt.float32
BF16 = mybir.dt.bfloat16
F8 = mybir.dt.float8e4
U8 = mybir.dt.uint8
EXP = mybir.ActivationFunctionType.Exp
COPY = mybir.ActivationFunctionType.Copy
DR = mybir.MatmulPerfMode.DoubleRow

P = 128
N = 1024          # x1 rows
M = 1024          # x2 rows
D = 512           # feature dim
NT, MT, DC = N // P, M // P, D // P
NEGB = -30000.0   # exp(x + NEGB) == 0.0 exactly for |x| < 80
SX = 32.0         # x1w3 prescale (keeps bf16 products well-scaled)
SQ = 1.0          # q2c fp8 denormal error is negligible (~0.1% global)
LN4 = 1.3862943611198906

N_CORES = 8

_CACHE = {}


def _chunks(width, step=512):
    out = []
    o = 0
    while o < width:
        w = min(step, width - o)
        out.append((o, w))
        o += w
    return out


def _pairs(k):
    """(k0, is_pair) covering range(k) with DoubleRow pairs + odd tail."""
    out = [(2 * i, True) for i in range(k // 2)]
    if k % 2:
        out.append((k - 1, False))
    return out


def _build(kn, km):
    vm = km * P
    nc = bacc.Bacc("TRN2", target_bir_lowering=False, debug=False)
    x1d = nc.dram_tensor("x1", [P, NT * D], F32, kind="ExternalInput").ap()
    x2d = nc.dram_tensor("x2", [P, MT * D], F32, kind="ExternalInput").ap()
    m1d = nc.dram_tensor("x1_mask", [N], U8, kind="ExternalInput").ap()
    m2d = nc.dram_tensor("x2_mask", [M], U8, kind="ExternalInput").ap()
    wd = nc.dram_tensor("W", [3 * D], F32, kind="ExternalInput").ap()
    o12d = nc.dram_tensor("out12", [P, NT * 2 * D], F32,
                          kind="ExternalOutput").ap()
    o3d = nc.dram_tensor("out3", [P, NT * D], F32, kind="ExternalOutput").ap()

    x1r_d = x1d.rearrange("p (t d) -> p t d", t=NT)
    x2r_d = x2d.rearrange("p (t d) -> p t d", t=MT)
    o12_r = o12d.rearrange("p (t e) -> p t e", t=NT)
    o3_r = o3d.rearrange("p (t e) -> p t e", t=NT)

    with tile.TileContext(nc) as tc, ExitStack() as ctx:
        big = ctx.enter_context(tc.tile_pool(name="big", bufs=1))
        rows = ctx.enter_context(tc.tile_pool(name="rows", bufs=1))
        work = ctx.enter_context(tc.tile_pool(name="work", bufs=4))
        psb = ctx.enter_context(tc.tile_pool(name="psb", bufs=2, space="PSUM"))
        psu = ctx.enter_context(tc.tile_pool(name="psu", bufs=3, space="PSUM"))
        psd = ctx.enter_context(tc.tile_pool(name="psd", bufs=1, space="PSUM"))

        # ---------------- constants ----------------
        ident = big.tile([P, P], F32)
        make_identity(nc, ident)
        identb = big.tile([P, P], BF16)
        nc.vector.tensor_copy(identb[:], ident[:])
        negln4 = big.tile([P, 1], F32)
        nc.vector.memset(negln4[:], -LN4)

        # ---------------- DMA loads (split SP / Act queues) ----------------
        wrow = rows.tile([1, 12 * P], F32)
        nc.scalar.dma_start(wrow[:], wd.rearrange("(a n) -> a n", a=1))
        x1n = big.tile([P, NT, D], F32)
        x2n = big.tile([P, km, D], F32)
        nc.sync.dma_start(x2n[:, 0:min(4, km), :], x2r_d[:, 0:min(4, km), :])
        nc.scalar.dma_start(x1n[:, 0:4, :], x1r_d[:, 0:4, :])
        if km > 4:
            nc.sync.dma_start(x2n[:, 4:km, :], x2r_d[:, 4:km, :])
        nc.scalar.dma_start(x1n[:, 4:8, :], x1r_d[:, 4:8, :])
        m1row = rows.tile([1, N], U8)
        nc.sync.dma_start(m1row[:], m1d.rearrange("(a n) -> a n", a=1))
        m2row = rows.tile([1, M], U8)
        nc.sync.dma_start(m2row[:], m2d.rearrange("(a n) -> a n", a=1))

        # ---------------- PE warmup (keeps the HAM clock busy) -------------
        # ~10 fat dummy matmuls (512 cyc each) bridge the load wait so the
        # HAM un-throttles before the real transposes start.
        wscr = big.tile([P, D], BF16)
        nc.vector.memset(wscr[:], 0.25)
        for i in range(16):
            pw = psb.tile([P, 1024], F32, tag="ps_sim", name=f"warm{i}")
            nc.tensor.matmul(pw[:, 0:D], identb[:], wscr[:], start=True,
                             stop=True)

        # ---------------- W prep ----------------
        pwc = psd.tile([P, 16], F32, tag="small", name="pwc")
        for c in range(12):
            nc.tensor.transpose(pwc[:, c:c + 1], wrow[0:1, c * P:(c + 1) * P],
                                ident[0:1, 0:1])
        wcols = big.tile([P, 12], F32)
        nc.vector.tensor_copy(wcols[:], pwc[:, 0:12])
        w3rec = big.tile([P, 4], F32)
        nc.vector.reciprocal(w3rec[:], wcols[:, 8:12])
        u1f = big.tile([P, 4], F32)
        nc.vector.tensor_mul(u1f[:], wcols[:, 0:4], w3rec[:])
        u1r = big.tile([P, 4], BF16)       # w1/(w3*SX): recovers s1 from x1w3T
        nc.vector.tensor_scalar_mul(u1r[:], u1f[:], 1.0 / SX)
        w2r = big.tile([P, 4], BF16)
        nc.vector.tensor_copy(w2r[:], wcols[:, 4:8])
        w3s = big.tile([P, 4], F32)        # w3 * SX (x1w3T eviction scale)
        nc.vector.tensor_scalar_mul(w3s[:], wcols[:, 8:12], SX)

        logm1 = rows.tile([1, N], F32)
        nc.vector.tensor_scalar_mul(logm1[:], m1row[:], NEGB)
        logm2 = rows.tile([1, M], F32)
        nc.vector.tensor_scalar_mul(logm2[:], m2row[:], NEGB)

        # ---------------- PE transposes of x1 / x2 ----------------
        x1w3T = big.tile([P, NT, DC, P], BF16)   # (d_lo, t, c, n_lo) = x1*w3*SX
        x2T = big.tile([P, km, DC, P], BF16)     # (d_lo, u, c, m_lo)

        def xpose_group(src, dst, q, jw, c, scale, nm):
            """Transpose tiles q*4..q*4+jw of src at d-chunk c -> dst cols."""
            pq = psu.tile([P, 4 * P], F32, tag="ps_uv", name=f"xp{nm}{q}_{c}")
            for j in range(jw):
                nc.tensor.transpose(pq[:, j * P:(j + 1) * P],
                                    src[:, q * 4 + j, c * P:(c + 1) * P],
                                    ident[:])
            out_ap = dst[:, q * 4:q * 4 + jw, c, :]
            if scale is None:
                nc.scalar.activation(out_ap, pq[:, 0:jw * P], COPY)
            else:
                nc.vector.tensor_scalar_mul(out_ap, pq[:, 0:jw * P],
                                            scale[:, c:c + 1])

        def s_chunk(name, lhs, rhsT, brow, logm, off, w):
            t0, ntile = off // P, w // P
            ps_s = psd.tile([1, D], F32, tag="small", name=f"ps{name}{off}")
            for c in range(DC):
                nc.tensor.matmul(ps_s[0:1, 0:w], lhs[:, c:c + 1],
                                 rhsT[:, t0:t0 + ntile, c, :],
                                 start=(c == 0), stop=(c == DC - 1))
            nc.vector.tensor_add(brow[:, off:off + w], ps_s[0:1, 0:w],
                                 logm[:, off:off + w])

        def col_of(name, brow, nt):
            pbc = psd.tile([P, 16], F32, tag="small", name=f"pbc{name}")
            for t in range(nt):
                nc.tensor.transpose(pbc[:, t:t + 1], brow[0:1, t * P:(t + 1) * P],
                                    ident[0:1, 0:1])
            return pbc

        b1row = rows.tile([1, N], F32)
        b2row = rows.tile([1, M], F32)
        E_raw = big.tile([P, NT, vm], BF16)      # exp(sim), n-major
        ETraw = big.tile([P, NT, km, P], BF16)   # (m_lo, t, u, n_lo)
        ET8 = big.tile([P, NT, km, P], F8)
        x1aug = big.tile([P, kn, D], BF16)       # x1 * g1
        x2aug = big.tile([P, km, D], F8)         # x2 * g2/4
        mch = _chunks(vm)

        def sim_tile(t):
            ps = psb.tile([P, 1024], F32, tag="ps_sim", name=f"sim{t}")
            for off, w in mch:
                u0, nu = off // P, w // P
                for c in range(DC):
                    nc.tensor.matmul(ps[:, off:off + w],
                                     x1w3T[:, t, c, :],
                                     x2T[:, u0:u0 + nu, c, :],
                                     start=(c == 0), stop=(c == DC - 1))
            nc.scalar.activation(E_raw[:, t, :], ps[:, 0:vm], EXP, scale=1.0 / SX)
            nc.sync.dma_start_transpose(ETraw[:, t, :, :], E_raw[:, t, :])
            nc.vector.tensor_copy(ET8[:, t, :, :], ETraw[:, t, :, :])

        # x2 transposes, then s2 -> g2c4 -> x2aug while the Act head is idle
        for c in range(DC):
            xpose_group(x2n, x2T, 0, min(4, km), c, None, "x2")
        if km > 4:
            for c in range(DC):
                xpose_group(x2n, x2T, 1, km - 4, c, None, "x2")
        for off, w in _chunks(vm):
            s_chunk("b2", w2r, x2T, b2row, logm2, off, w)
        pbc2 = col_of("b2", b2row, km)
        g2c4 = big.tile([P, km], F32)     # exp(s2 + logm2 - ln4) = g2/4
        nc.scalar.activation(g2c4[:], pbc2[:, 0:km], EXP, bias=negln4[:, 0:1])
        g2c8 = big.tile([P, 8, 16], F8)
        for u in range(km):
            nc.vector.tensor_copy(g2c8[:, u, 0:1], g2c4[:, u:u + 1])
        for u in range(km):
            nc.scalar.activation(x2aug[:, u, :], x2n[:, u, :], COPY,
                                 scale=g2c4[:, u:u + 1])

        # x1 transposes quad 0, first sim tiles, quad 1, s1, rest of sim
        for c in range(DC):
            xpose_group(x1n, x1w3T, 0, 4, c, w3s, "x1")
        for t in range(4):
            sim_tile(t)
        for c in range(DC):
            xpose_group(x1n, x1w3T, 1, 4, c, w3s, "x1")
        for off, w in _chunks(N):
            s_chunk("b1", u1r, x1w3T, b1row, logm1, off, w)
        pbc1 = col_of("b1", b1row, NT)
        g1c = big.tile([P, NT], F32)      # exp(s1 + logm1)
        nc.scalar.activation(g1c[:], pbc1[:, 0:NT], EXP)
        g1cb = big.tile([P, 8, 16], BF16)
        for k in range(kn):
            nc.vector.tensor_copy(g1cb[:, k, 0:1], g1c[:, k:k + 1])
        for t in range(kn):
            nc.vector.tensor_scalar_mul(x1aug[:, t, :], x1n[:, t, :],
                                        g1c[:, t:t + 1])
        for t in range(4, NT):
            sim_tile(t)

        # ---------------- den2 -> rden2/rQ ----------------
        kp_m = _pairs(km)
        den2row = rows.tile([1, vm], F32)
        for off, w in _chunks(vm):
            ps_d = psd.tile([1, D], F32, tag="small", name=f"psden2{off}")
            for k in range(kn):
                nc.tensor.matmul(ps_d[0:1, 0:w], g1cb[:, k, 0:1],
                                 E_raw[:, k, off:off + w],
                                 start=(k == 0), stop=(k == kn - 1))
            nc.scalar.activation(den2row[:, off:off + w], ps_d[0:1, 0:w], COPY)
        pdc2 = col_of("d2", den2row, km)
        rden2 = big.tile([P, km], F32)
        nc.vector.reciprocal(rden2[:], pdc2[:, 0:km])
        rQ = big.tile([P, km], F32)          # rden2 * g2/4
        nc.vector.tensor_mul(rQ[:], rden2[:], g2c4[:])

        # ---------------- den1 -> rden1 ----------------
        den1row = rows.tile([1, N], F32)
        for t in range(NT):
            ps_d = psd.tile([1, D], F32, tag="small", name=f"psden1{t}")
            for i, (k0, pair) in enumerate(kp_m):
                last = i == len(kp_m) - 1
                if pair:
                    nc.tensor.matmul(ps_d[0:1, 0:P], g2c8[:, k0:k0 + 2, 0:1],
                                     ET8[:, t, k0:k0 + 2, :],
                                     start=(i == 0), stop=last, perf_mode=DR)
                else:
                    nc.tensor.matmul(ps_d[0:1, 0:P], g2c8[:, k0, 0:1],
                                     ET8[:, t, k0, :], start=(i == 0), stop=last)
            nc.scalar.activation(den1row[:, t * P:(t + 1) * P], ps_d[0:1, 0:P],
                                 COPY)
        pdc1 = col_of("d1", den1row, NT)
        rden1 = big.tile([P, NT], F32)
        nc.vector.reciprocal(rden1[:], pdc1[:, 0:NT])
        # x1 pre-scale overlaps U_col on DVE (blocks 2 and 3 share it since
        # SQ == 1)
        x1rd = big.tile([P, NT, D], F32)     # x1 * rden1
        for t in range(NT):
            nc.vector.tensor_scalar_mul(x1rd[:, t, :], x1n[:, t, :],
                                        rden1[:, t:t + 1])

        # ---------------- U_col -> Q2C ----------------
        Q2C = big.tile([P, km, D], F8)       # q2c * g2/4 * SQ
        for u in range(km):
            pu = psu.tile([P, D], F32, tag="ps_uv", name=f"pu{u}")
            for k in range(kn):
                nc.tensor.matmul(pu[:], E_raw[:, k, u * P:(u + 1) * P],
                                 x1aug[:, k, :], start=(k == 0),
                                 stop=(k == kn - 1))
            nc.scalar.activation(Q2C[:, u, :], pu[:], COPY, scale=rQ[:, u:u + 1])

        # ---------------- U_row -> blocks 1+2 ; V -> block 3 ----------------
        def uv_mm(ps_ap, t, rhs):
            for i, (k0, pair) in enumerate(kp_m):
                last = i == len(kp_m) - 1
                if pair:
                    nc.tensor.matmul(ps_ap, ET8[:, t, k0:k0 + 2, :],
                                     rhs[:, k0:k0 + 2, :],
                                     start=(i == 0), stop=last, perf_mode=DR)
                else:
                    nc.tensor.matmul(ps_ap, ET8[:, t, k0, :], rhs[:, k0, :],
                                     start=(i == 0), stop=last)

        for t in range(NT):
            if t % 2 == 0:
                c12 = work.tile([P, 2, 2 * D], F32, tag="ev", name=f"c12_{t}")
            half = c12[:, t % 2, :]
            if t % 2 == 0:
                pr = psu.tile([P, D], F32, tag="ps_uv", name=f"pr{t}")
            else:
                prb = psb.tile([P, 1024], F32, tag="ps_sim", name=f"prb{t}")
                pr = prb[:, 0:D]
            uv_mm(pr[:], t, x2aug)
            nc.scalar.activation(half[:, 0:D], pr[:], COPY,
                                 scale=rden1[:, t:t + 1])
            nc.vector.tensor_mul(half[:, D:2 * D], x1rd[:, t, :], pr[:])
            if t % 2 == 1:
                eng = nc.sync if t % 4 == 1 else nc.scalar
                eng.dma_start(o12_r[:, t - 1:t + 1, :], c12[:])

        for t in range(NT):
            if t % 4 == 0:
                o3t = work.tile([P, 4, D], F32, tag="o3", name=f"o3_{t}")
            if t % 2 == 0:
                pv = psu.tile([P, D], F32, tag="ps_uv", name=f"pv{t}")
            else:
                pvb = psb.tile([P, 1024], F32, tag="ps_sim", name=f"pvb{t}")
                pv = pvb[:, 0:D]
            uv_mm(pv[:], t, Q2C)
            nc.vector.tensor_mul(o3t[:, t % 4, :], x1rd[:, t, :], pv[:])
            if t % 4 == 3:
                eng = nc.sync if t == 3 else nc.scalar
                eng.dma_start(o3_r[:, t - 3:t + 1, :], o3t[:])

    nc.compile()
    return nc


def _kept_tiles(mask):
    """Tiles (of 128) up to and including the last one with any valid row."""
    valid = ~mask.astype(bool)
    any_valid = valid.reshape(valid.shape[0], -1, P).any(axis=2).any(axis=0)
    nz = np.nonzero(any_valid)[0]
    return int(nz[-1]) + 1 if len(nz) else 1


def _get_nc(kn, km):
    key = (kn, km)
    if key not in _CACHE:
        _CACHE[key] = _build(kn, km)
    return _CACHE[key]


def _run(inputs, trace=False, trace_cores=None):
    x1 = np.ascontiguousarray(np.asarray(inputs["x1"], dtype=np.float32))
    x2 = np.ascontiguousarray(np.asarray(inputs["x2"], dtype=np.float32))
    m1 = np.ascontiguousarray(np.asarray(inputs["x1_mask"]).astype(np.uint8))
    m2 = np.ascontiguousarray(np.asarray(inputs["x2_mask"]).astype(np.uint8))
    W = np.ascontiguousarray(np.asarray(inputs["W"], dtype=np.float32))
    nc = _get_nc(_kept_tiles(m1), _kept_tiles(m2))
    # partition-major device layouts: per-partition rows are 8-16 KB
    # contiguous, so each load/store is ~128 fat DMA descriptors instead of
    # thousands of 2 KB ones (a single HWDGE queue only sustains ~100 GB/s
    # on 2 KB descriptors).
    x1p = np.ascontiguousarray(
        x1.reshape(N_CORES, NT, P, D).transpose(0, 2, 1, 3).reshape(
            N_CORES, P, NT * D))
    x2p = np.ascontiguousarray(
        x2.reshape(N_CORES, MT, P, D).transpose(0, 2, 1, 3).reshape(
            N_CORES, P, MT * D))
    in_maps = [
        {"x1": x1p[i], "x2": x2p[i], "x1_mask": m1[i], "x2_mask": m2[i],
         "W": W}
        for i in range(N_CORES)
    ]
    res = run_bass_kernel_spmd(nc, in_maps, core_ids=list(range(N_CORES)),
                               trace=trace, trace_cores=trace_cores)
    # device returns blocks 1+2 and 3 (partition-major); block 0 is x1
    out = np.empty((N_CORES, N, 4 * D), dtype=np.float32)
    out[:, :, 0:D] = x1
    for i in range(N_CORES):
        d12 = res.results[i]["out12"].reshape(P, NT, 2 * D)
        out[i, :, D:3 * D] = d12.transpose(1, 0, 2).reshape(N, 2 * D)
        d3 = res.results[i]["out3"].reshape(P, NT, D)
        out[i, :, 3 * D:] = d3.transpose(1, 0, 2).reshape(N, D)
    return out, res


def kernel(x1, x1_mask, x2, x2_mask, W, bias=None, **_kw):
    # bias is mathematically irrelevant: a global additive constant cancels in
    # both softmaxes, and every output term is softmax-weighted.
    out, _ = _run({"x1": x1, "x1_mask": x1_mask, "x2": x2, "x2_mask": x2_mask,
                   "W": W})
    return out


# revision 15
# speedup vs baseline: 1.1966x; 1.0309x over previous
"""Trainium2 Bass kernel for BiAttention (b=8, n=m=1024, d=512).

Sharding: data-parallel over batch — one batch element per NeuronCore,
8 cores, no cross-core communication.

Per-core algorithm (softmax shift-invariance folds the Linear(3d,1)
row/col terms, bias, and both padding masks into per-row/col exponent
weights g1 = exp(s1+logm1), g2 = exp(s2+logm2); logits ~ N(0,1) so raw
exp is safe):

  sim      = (x1*w3) @ x2^T              (n, m)   [tri term only]
  E        = exp(sim)                    bf16
  ET8      = fp8(E^T)                    via DMA xbar transpose + DVE cast
  U_row    = ET8^T @ (x2*g2/4)  -> c2q = U_row/den1,  den1 = g2c8 @ ET8
  U_col    = E^T   @ (x1*g1)    -> q2c = U_col/den2,  den2 = g1cb @ E
  V        = ET8^T @ Q2C        -> q2c_att = V * rden1/SQ
  out      = [x1, c2q, x1*c2q, x1*q2c_att]        (n, 4d)

Precision: sim/U_col in bf16, U_row/V in fp8e4 DoubleRow (2 contraction
tiles per instruction), f32 PSUM accumulation, exact f32 softmax
divisions.  Measured end-to-end rel err ~3e-3 (gate 2e-2).

Engine/DMA layout (calibrated on HW):
  - PE transposes x1/x2 directly from f32 (2 cyc/row) during the load
    phase — keeps the HAM clock warm and removes conversion latency; the
    mid-kernel E transposes use the DMA xbar (bf16) instead so the PE
    stays on matmuls.
  - PSUM evictions split Act/DVE by phase; gpsimd (no PSUM access,
    ~2.5 ns/el) only gets SBUF->SBUF scales/muls in its idle phases.
  - Loads split across the SP and Act HWDGE queues (a single queue
    sustains only ~133 GB/s on the 2 KB-granule input layout).  Output
    blocks 1-3 are staged contiguously per tile (6 KB/partition rows)
    and stored after an interleaved U_row/V loop, alternating queues.

Mask-suffix specialization: 128-row tiles fully masked at the end of
either sequence are skipped in the contractions (host inspects masks and
dispatches to a NEFF compiled for that (kn, km)); partially-masked tiles
are exact via the exponent weights.
"""

import numpy as np
from contextlib import ExitStack

import concourse.bacc as bacc
import concourse.tile as tile
import concourse.mybir as mybir
from concourse.bass_utils import run_bass_kernel_spmd
from concourse.masks import make_identity

F32 = mybir.dt.float32
BF16 = mybir.dt.bfloat16
F8 = mybir.dt.float8e4
U8 = mybir.dt.uint8
EXP = mybir.ActivationFunctionType.Exp
COPY = mybir.ActivationFunctionType.Copy
DR = mybir.MatmulPerfMode.DoubleRow

P = 128
N = 1024          # x1 rows
M = 1024          # x2 rows
D = 512           # feature dim
NT, MT, DC = N // P, M // P, D // P
NEGB = -30000.0   # exp(x + NEGB) == 0.0 exactly for |x| < 80
SX = 32.0         # x1w3 prescale (keeps bf16 products well-scaled)
SQ = 1.0          # q2c fp8 denormal error is negligible (~0.1% global)
LN4 = 1.3862943611198906

N_CORES = 8

_CACHE = {}


def _chunks(width, step=512):
    out = []
    o = 0
    while o < width:
        w = min(step, width - o)
        out.append((o, w))
        o += w
    return out


def _pairs(k):
    """(k0, is_pair) covering range(k) with DoubleRow pairs + odd tail."""
    out = [(2 * i, True) for i in range(k // 2)]
    if k % 2:
        out.append((k - 1, False))
    return out


def _build(kn, km):
    vm = km * P
    nc = bacc.Bacc("TRN2", target_bir_lowering=False, debug=False)
    x1d = nc.dram_tensor("x1", [P, NT * D], F32, kind="ExternalInput").ap()
    x2d = nc.dram_tensor("x2", [P, MT * D], F32, kind="ExternalInput").ap()
    m1d = nc.dram_tensor("x1_mask", [N], U8, kind="ExternalInput").ap()
    m2d = nc.dram_tensor("x2_mask", [M], U8, kind="ExternalInput").ap()
    wd = nc.dram_tensor("W", [3 * D], F32, kind="ExternalInput").ap()
    o12d = nc.dram_tensor("out12", [P, NT * 2 * D], F32,
                          kind="ExternalOutput").ap()
    o3d = nc.dram_tensor("out3", [P, NT * D], F32, kind="ExternalOutput").ap()

    x1r_d = x1d.rearrange("p (t d) -> p t d", t=NT)
    x2r_d = x2d.rearrange("p (t d) -> p t d", t=MT)
    o12_r = o12d.rearrange("p (t e) -> p t e", t=NT)
    o3_r = o3d.rearrange("p (t e) -> p t e", t=NT)

    with tile.TileContext(nc) as tc, ExitStack() as ctx:
        big = ctx.enter_context(tc.tile_pool(name="big", bufs=1))
        rows = ctx.enter_context(tc.tile_pool(name="rows", bufs=1))
        work = ctx.enter_context(tc.tile_pool(name="work", bufs=4))
        psb = ctx.enter_context(tc.tile_pool(name="psb", bufs=2, space="PSUM"))
        psu = ctx.enter_context(tc.tile_pool(name="psu", bufs=3, space="PSUM"))
        psd = ctx.enter_context(tc.tile_pool(name="psd", bufs=1, space="PSUM"))

        # ---------------- constants ----------------
        ident = big.tile([P, P], F32)
        make_identity(nc, ident)
        identb = big.tile([P, P], BF16)
        nc.vector.tensor_copy(identb[:], ident[:])
        negln4 = big.tile([P, 1], F32)
        nc.vector.memset(negln4[:], -LN4)

        # ---------------- DMA loads (split SP / Act queues) ----------------
        wrow = rows.tile([1, 12 * P], F32)
        nc.scalar.dma_start(wrow[:], wd.rearrange("(a n) -> a n", a=1))
        x1n = big.tile([P, NT, D], F32)
        x2n = big.tile([P, km, D], F32)
        nc.sync.dma_start(x2n[:, 0:min(4, km), :], x2r_d[:, 0:min(4, km), :])
        nc.scalar.dma_start(x1n[:, 0:4, :], x1r_d[:, 0:4, :])
        if km > 4:
            nc.sync.dma_start(x2n[:, 4:km, :], x2r_d[:, 4:km, :])
        nc.scalar.dma_start(x1n[:, 4:8, :], x1r_d[:, 4:8, :])
        m1row = rows.tile([1, N], U8)
        nc.sync.dma_start(m1row[:], m1d.rearrange("(a n) -> a n", a=1))
        m2row = rows.tile([1, M], U8)
        nc.sync.dma_start(m2row[:], m2d.rearrange("(a n) -> a n", a=1))

        # ---------------- PE warmup (keeps the HAM clock busy) -------------
        # ~10 fat dummy matmuls (512 cyc each) bridge the load wait so the
        # HAM un-throttles before the real transposes start.
        wscr = big.tile([P, D], BF16)
        nc.vector.memset(wscr[:], 0.25)
        for i in range(16):
            pw = psb.tile([P, 1024], F32, tag="ps_sim", name=f"warm{i}")
            nc.tensor.matmul(pw[:, 0:D], identb[:], wscr[:], start=True,
                             stop=True)

        # ---------------- W prep ----------------
        pwc = psd.tile([P, 16], F32, tag="small", name="pwc")
        for c in range(12):
            nc.tensor.transpose(pwc[:, c:c + 1], wrow[0:1, c * P:(c + 1) * P],
                                ident[0:1, 0:1])
        wcols = big.tile([P, 12], F32)
        nc.vector.tensor_copy(wcols[:], pwc[:, 0:12])
        w3rec = big.tile([P, 4], F32)
        nc.vector.reciprocal(w3rec[:], wcols[:, 8:12])
        u1f = big.tile([P, 4], F32)
        nc.vector.tensor_mul(u1f[:], wcols[:, 0:4], w3rec[:])
        u1r = big.tile([P, 4], BF16)       # w1/(w3*SX): recovers s1 from x1w3T
        nc.vector.tensor_scalar_mul(u1r[:], u1f[:], 1.0 / SX)
        w2r = big.tile([P, 4], BF16)
        nc.vector.tensor_copy(w2r[:], wcols[:, 4:8])
        w3s = big.tile([P, 4], F32)        # w3 * SX (x1w3T eviction scale)
        nc.vector.tensor_scalar_mul(w3s[:], wcols[:, 8:12], SX)

        logm1 = rows.tile([1, N], F32)
        nc.vector.tensor_scalar_mul(logm1[:], m1row[:], NEGB)
        logm2 = rows.tile([1, M], F32)
        nc.vector.tensor_scalar_mul(logm2[:], m2row[:], NEGB)

        # ---------------- PE transposes of x1 / x2 ----------------
        x1w3T = big.tile([P, NT, DC, P], BF16)   # (d_lo, t, c, n_lo) = x1*w3*SX
        x2T = big.tile([P, km, DC, P], BF16)     # (d_lo, u, c, m_lo)

        def xpose_group(src, dst, q, jw, c, scale, nm):
            """Transpose tiles q*4..q*4+jw of src at d-chunk c -> dst cols."""
            pq = psu.tile([P, 4 * P], F32, tag="ps_uv", name=f"xp{nm}{q}_{c}")
            for j in range(jw):
                nc.tensor.transpose(pq[:, j * P:(j + 1) * P],
                                    src[:, q * 4 + j, c * P:(c + 1) * P],
                                    ident[:])
            out_ap = dst[:, q * 4:q * 4 + jw, c, :]
            if scale is None:
                nc.scalar.activation(out_ap, pq[:, 0:jw * P], COPY)
            else:
                nc.vector.tensor_scalar_mul(out_ap, pq[:, 0:jw * P],
                                            scale[:, c:c + 1])

        def s_chunk(name, lhs, rhsT, brow, logm, off, w):
            t0, ntile = off // P, w // P
            ps_s = psd.tile([1, D], F32, tag="small", name=f"ps{name}{off}")
            for c in range(DC):
                nc.tensor.matmul(ps_s[0:1, 0:w], lhs[:, c:c + 1],
                                 rhsT[:, t0:t0 + ntile, c, :],
                                 start=(c == 0), stop=(c == DC - 1))
            nc.vector.tensor_add(brow[:, off:off + w], ps_s[0:1, 0:w],
                                 logm[:, off:off + w])

        def col_of(name, brow, nt):
            pbc = psd.tile([P, 16], F32, tag="small", name=f"pbc{name}")
            for t in range(nt):
                nc.tensor.transpose(pbc[:, t:t + 1], brow[0:1, t * P:(t + 1) * P],
                                    ident[0:1, 0:1])
            return pbc

        b1row = rows.tile([1, N], F32)
        b2row = rows.tile([1, M], F32)
        E_raw = big.tile([P, NT, vm], BF16)      # exp(sim), n-major
        ETraw = big.tile([P, NT, km, P], BF16)   # (m_lo, t, u, n_lo)
        ET8 = big.tile([P, NT, km, P], F8)
        x1aug = big.tile([P, kn, D], BF16)       # x1 * g1
        x2aug = big.tile([P, km, D], F8)         # x2 * g2/4
        mch = _chunks(vm)

        def sim_tile(t):
            ps = psb.tile([P, 1024], F32, tag="ps_sim", name=f"sim{t}")
            for off, w in mch:
                u0, nu = off // P, w // P
                for c in range(DC):
                    nc.tensor.matmul(ps[:, off:off + w],
                                     x1w3T[:, t, c, :],
                                     x2T[:, u0:u0 + nu, c, :],
                                     start=(c == 0), stop=(c == DC - 1))
            nc.scalar.activation(E_raw[:, t, :], ps[:, 0:vm], EXP, scale=1.0 / SX)
            nc.sync.dma_start_transpose(ETraw[:, t, :, :], E_raw[:, t, :])
            nc.vector.tensor_copy(ET8[:, t, :, :], ETraw[:, t, :, :])

        # x2 transposes, then s2 -> g2c4 -> x2aug while the Act head is idle
        for c in range(DC):
            xpose_group(x2n, x2T, 0, min(4, km), c, None, "x2")
        if km > 4:
            for c in range(DC):
                xpose_group(x2n, x2T, 1, km - 4, c, None, "x2")
        for off, w in _chunks(vm):
            s_chunk("b2", w2r, x2T, b2row, logm2, off, w)
        pbc2 = col_of("b2", b2row, km)
        g2c4 = big.tile([P, km], F32)     # exp(s2 + logm2 - ln4) = g2/4
        nc.scalar.activation(g2c4[:], pbc2[:, 0:km], EXP, bias=negln4[:, 0:1])
        g2c8 = big.tile([P, 8, 16], F8)
        for u in range(km):
            nc.vector.tensor_copy(g2c8[:, u, 0:1], g2c4[:, u:u + 1])
        for u in range(km):
            nc.scalar.activation(x2aug[:, u, :], x2n[:, u, :], COPY,
                                 scale=g2c4[:, u:u + 1])

        # x1 transposes quad 0, first sim tiles, quad 1, s1, rest of sim
        for c in range(DC):
            xpose_group(x1n, x1w3T, 0, 4, c, w3s, "x1")
        for t in range(4):
            sim_tile(t)
        for c in range(DC):
            xpose_group(x1n, x1w3T, 1, 4, c, w3s, "x1")
        for off, w in _chunks(N):
            s_chunk("b1", u1r, x1w3T, b1row, logm1, off, w)
        pbc1 = col_of("b1", b1row, NT)
        g1c = big.tile([P, NT], F32)      # exp(s1 + logm1)
        nc.scalar.activation(g1c[:], pbc1[:, 0:NT], EXP)
        g1cb = big.tile([P, 8, 16], BF16)
        for k in range(kn):
            nc.vector.tensor_copy(g1cb[:, k, 0:1], g1c[:, k:k + 1])
        for t in range(kn):
            nc.vector.tensor_scalar_mul(x1aug[:, t, :], x1n[:, t, :],
                                        g1c[:, t:t + 1])
        for t in range(4, NT):
            sim_tile(t)

        # ---------------- den1 -> rden1 (ET8 is ready at sim end) ----
        kp_m = _pairs(km)
        den1row = rows.tile([1, N], F32)
        for t in range(NT):
            ps_d = psd.tile([1, D], F32, tag="small", name=f"psden1{t}")
            for i, (k0, pair) in enumerate(kp_m):
                last = i == len(kp_m) - 1
                if pair:
                    nc.tensor.matmul(ps_d[0:1, 0:P], g2c8[:, k0:k0 + 2, 0:1],
                                     ET8[:, t, k0:k0 + 2, :],
                                     start=(i == 0), stop=last, perf_mode=DR)
                else:
                    nc.tensor.matmul(ps_d[0:1, 0:P], g2c8[:, k0, 0:1],
                                     ET8[:, t, k0, :], start=(i == 0), stop=last)
            nc.scalar.activation(den1row[:, t * P:(t + 1) * P], ps_d[0:1, 0:P],
                                 COPY)
        pdc1 = col_of("d1", den1row, NT)
        rden1 = big.tile([P, NT], F32)
        nc.vector.reciprocal(rden1[:], pdc1[:, 0:NT])
        # x1 pre-scale overlaps U_col on DVE (blocks 2 and 3 share it since
        # SQ == 1)
        x1rd = big.tile([P, NT, D], F32)     # x1 * rden1
        for t in range(NT):
            nc.vector.tensor_scalar_mul(x1rd[:, t, :], x1n[:, t, :],
                                        rden1[:, t:t + 1])

        # ---------------- den2 -> rden2/rQ ----------------
        den2row = rows.tile([1, vm], F32)
        for off, w in _chunks(vm):
            ps_d = psd.tile([1, D], F32, tag="small", name=f"psden2{off}")
            for k in range(kn):
                nc.tensor.matmul(ps_d[0:1, 0:w], g1cb[:, k, 0:1],
                                 E_raw[:, k, off:off + w],
                                 start=(k == 0), stop=(k == kn - 1))
            nc.scalar.activation(den2row[:, off:off + w], ps_d[0:1, 0:w], COPY)
        pdc2 = col_of("d2", den2row, km)
        rden2 = big.tile([P, km], F32)
        nc.vector.reciprocal(rden2[:], pdc2[:, 0:km])
        rQ = big.tile([P, km], F32)          # rden2 * g2/4
        nc.vector.tensor_mul(rQ[:], rden2[:], g2c4[:])

        # ---------------- U_col -> Q2C ----------------
        Q2C = big.tile([P, km, D], F8)       # q2c * g2/4 * SQ
        for u in range(km):
            pu = psu.tile([P, D], F32, tag="ps_uv", name=f"pu{u}")
            for k in range(kn):
                nc.tensor.matmul(pu[:], E_raw[:, k, u * P:(u + 1) * P],
                                 x1aug[:, k, :], start=(k == 0),
                                 stop=(k == kn - 1))
            nc.scalar.activation(Q2C[:, u, :], pu[:], COPY, scale=rQ[:, u:u + 1])

        # ---------------- U_row -> blocks 1+2 ; V -> block 3 ----------------
        def uv_mm(ps_ap, t, rhs):
            for i, (k0, pair) in enumerate(kp_m):
                last = i == len(kp_m) - 1
                if pair:
                    nc.tensor.matmul(ps_ap, ET8[:, t, k0:k0 + 2, :],
                                     rhs[:, k0:k0 + 2, :],
                                     start=(i == 0), stop=last, perf_mode=DR)
                else:
                    nc.tensor.matmul(ps_ap, ET8[:, t, k0, :], rhs[:, k0, :],
                                     start=(i == 0), stop=last)

        for t in range(NT):
            if t % 2 == 0:
                c12 = work.tile([P, 2, 2 * D], F32, tag="ev", name=f"c12_{t}")
            half = c12[:, t % 2, :]
            if t % 2 == 0:
                pr = psu.tile([P, D], F32, tag="ps_uv", name=f"pr{t}")
            else:
                prb = psb.tile([P, 1024], F32, tag="ps_sim", name=f"prb{t}")
                pr = prb[:, 0:D]
            uv_mm(pr[:], t, x2aug)
            nc.scalar.activation(half[:, 0:D], pr[:], COPY,
                                 scale=rden1[:, t:t + 1])
            nc.vector.tensor_mul(half[:, D:2 * D], x1rd[:, t, :], pr[:])
            if t % 2 == 1:
                eng = nc.sync if t % 4 == 1 else nc.scalar
                eng.dma_start(o12_r[:, t - 1:t + 1, :], c12[:])

        for t in range(NT):
            if t % 4 == 0:
                o3t = work.tile([P, 4, D], F32, tag="o3", name=f"o3_{t}")
            if t % 2 == 0:
                pv = psu.tile([P, D], F32, tag="ps_uv", name=f"pv{t}")
            else:
                pvb = psb.tile([P, 1024], F32, tag="ps_sim", name=f"pvb{t}")
                pv = pvb[:, 0:D]
            uv_mm(pv[:], t, Q2C)
            nc.vector.tensor_mul(o3t[:, t % 4, :], x1rd[:, t, :], pv[:])
            if t % 4 == 3:
                eng = nc.sync if t == 3 else nc.scalar
                eng.dma_start(o3_r[:, t - 3:t + 1, :], o3t[:])

    nc.compile()
    return nc


def _kept_tiles(mask):
    """Tiles (of 128) up to and including the last one with any valid row."""
    valid = ~mask.astype(bool)
    any_valid = valid.reshape(valid.shape[0], -1, P).any(axis=2).any(axis=0)
    nz = np.nonzero(any_valid)[0]
    return int(nz[-1]) + 1 if len(nz) else 1


def _get_nc(kn, km):
    key = (kn, km)
    if key not in _CACHE:
        _CACHE[key] = _build(kn, km)
    return _CACHE[key]


def _run(inputs, trace=False, trace_cores=None):
    x1 = np.ascontiguousarray(np.asarray(inputs["x1"], dtype=np.float32))
    x2 = np.ascontiguousarray(np.asarray(inputs["x2"], dtype=np.float32))
    m1 = np.ascontiguousarray(np.asarray(inputs["x1_mask"]).astype(np.uint8))
    m2 = np.ascontiguousarray(np.asarray(inputs["x2_mask"]).astype(np.uint8))
    W = np.ascontiguousarray(np.asarray(inputs["W"], dtype=np.float32))
    nc = _get_nc(_kept_tiles(m1), _kept_tiles(m2))
    # partition-major device layouts: per-partition rows are 8-16 KB
    # contiguous, so each load/store is ~128 fat DMA descriptors instead of
    # thousands of 2 KB ones (a single HWDGE queue only sustains ~100 GB/s
    # on 2 KB descriptors).
    x1p = np.ascontiguousarray(
        x1.reshape(N_CORES, NT, P, D).transpose(0, 2, 1, 3).reshape(
            N_CORES, P, NT * D))
    x2p = np.ascontiguousarray(
        x2.reshape(N_CORES, MT, P, D).transpose(0, 2, 1, 3).reshape(
            N_CORES, P, MT * D))
    in_maps = [
        {"x1": x1p[i], "x2": x2p[i], "x1_mask": m1[i], "x2_mask": m2[i],
         "W": W}
        for i in range(N_CORES)
    ]
    res = run_bass_kernel_spmd(nc, in_maps, core_ids=list(range(N_CORES)),
                               trace=trace, trace_cores=trace_cores)
    # device returns blocks 1+2 and 3 (partition-major); block 0 is x1
    out = np.empty((N_CORES, N, 4 * D), dtype=np.float32)
    out[:, :, 0:D] = x1
    for i in range(N_CORES):
        d12 = res.results[i]["out12"].reshape(P, NT, 2 * D)
        out[i, :, D:3 * D] = d12.transpose(1, 0, 2).reshape(N, 2 * D)
        d3 = res.results[i]["out3"].reshape(P, NT, D)
        out[i, :, 3 * D:] = d3.transpose(1, 0, 2).reshape(N, D)
    return out, res


def kernel(x1, x1_mask, x2, x2_mask, W, bias=None, **_kw):
    # bias is mathematically irrelevant: a global additive constant cancels in
    # both softmaxes, and every output term is softmax-weighted.
    out, _ = _run({"x1": x1, "x1_mask": x1_mask, "x2": x2, "x2_mask": x2_mask,
                   "W": W})
    return out
